# revision 27
# baseline (speedup 1.0000x reference)
"""Trainium2 Bass kernel for 3D Haar wavelet transform (depthwise conv,
stride 2, kernel 2x2x2, 8-filter Haar bank per channel).

x: [2, 16, 128, 128, 128] f32  ->  y: [2, 128, 64, 64, 64] f32

Strategy (pure data parallel): the 32 (n, c) slabs are split 4-per-core
across 8 NeuronCores. Per slab [d=128, h=128, w=128], the separable Haar
transform is computed as:
  1. TensorE matmul with a fixed 128x128 butterfly matrix contracting the
     d partition axis -> (sum, diff) pairs over d, scale 1/8 folded in.
  2. ScalarE copy evicts PSUM -> SBUF (DVE tensor_tensor may read only one
     PSUM operand).
  3. VectorE (+ optionally GpSimd) add/sub over w pairs, then over h pairs,
     into per-(b,c) staging tiles laid out for contiguous output DMA.
All DMA transfers are >=1 MiB with >=8 KiB contiguous chunks.
"""

import sys

if "/opt/trn_rl_repo" not in sys.path:
    sys.path.insert(0, "/opt/trn_rl_repo")

import numpy as np

N_CORES = 8
SLABS = 4          # (n, c) slabs per core
D = 128
H = 128
WID = 128
HC = 16            # h-rows per chunk
NCHUNK = H // HC   # 8 chunks per slab
FREE = HC * WID    # 2048 f32 per partition per chunk
DH = D // 2        # 64
HH = H // 2
WH = WID // 2

# production configuration (HW A/B verified):
# io_dt="bf16": HBM transport in bf16 (x, W, y) — halves DMA bytes vs f32;
#   PE accumulates in fp32, Haar weights (+-0.125) are exact in bf16, and
#   rel err is ~5.8e-3 vs the 2e-2 gate.
# mm_deint: the matmul de-interleaves w-pairs via strided bf16 rhs views
#   (full rate: 4 hits per 16B SBUF line), so PSUM comes out (t, h, wh);
#   the ACT eviction is one contiguous copy and every DVE butterfly operand
#   is a packed 2-byte SBUF run (DVE 2x fast path).
# HW: 206.6us (f32) -> 136.8us (bf16) -> 109.8us (bf16 + mm_deint).
CFG = dict(
    f32r=False, perm_dh=True, in_batch=4, x_bufs=2, gps=0, st_split=2, st_bufs=1,
    st_eng="scalar", io_dt="bf16", chain_dt="bf16", mm_deint=True,
)


def _haar_weight_np() -> np.ndarray:
    lo = np.array([1.0, 1.0], dtype=np.float32) / 2
    hi = np.array([1.0, -1.0], dtype=np.float32) / 2
    filts = []
    for a in (lo, hi):
        for b in (lo, hi):
            for c in (lo, hi):
                filts.append(a[:, None, None] * b[None, :, None] * c[None, None, :])
    return np.stack(filts)


def _butterfly_lhsT(perm_dh: bool = False) -> np.ndarray:
    # lhsT[k, m]: matmul computes out[m, n] = sum_k lhsT[k, m] * rhs[k, n].
    # Output partition m encodes (a, dh): a=0 -> d-axis low-pass sum of planes
    # (2dh, 2dh+1), a=1 -> high-pass difference. perm_dh=False: m = a*64 + dh;
    # perm_dh=True: m = 2*dh + a (staging partitions ordered dh-major so one
    # 128-partition DMA covers both a halves).
    # The full 1/8 = (1/2)^3 scale of the separable transform is folded here
    # so the h/w stages are pure add/sub.
    b = np.zeros((128, 128), dtype=np.float32)
    f = np.float32(0.125)
    for j in range(64):
        m_lo = 2 * j if perm_dh else j
        m_hi = 2 * j + 1 if perm_dh else 64 + j
        b[2 * j, m_lo] = f
        b[2 * j + 1, m_lo] = f
        b[2 * j, m_hi] = f
        b[2 * j + 1, m_hi] = -f
    return b


def build_module(n_iters: int = 1, cfg: dict | None = None):
    """Build the per-core SPMD Bass module. n_iters > 1 wraps the whole body
    in a dynamic repeat loop (used only for timing measurements)."""
    import concourse.bacc as bacc
    import concourse.mybir as mybir
    import concourse.tile as tile
    from contextlib import ExitStack

    c = dict(CFG)
    if cfg:
        c.update(cfg)
    f32r = c["f32r"]
    perm_dh = c["perm_dh"]
    in_batch = c["in_batch"]
    x_bufs = c["x_bufs"]
    gps = c["gps"]
    st_split = c.get("st_split", 1)
    st_bufs = c.get("st_bufs", 2)
    cw_bufs = c.get("cw_bufs", 2)
    st_eng_name = c.get("st_eng", "scalar")
    io_bf16 = c.get("io_dt", "f32") == "bf16"
    split_evict = c.get("split_evict", False)
    h_merge = c.get("h_merge", False)
    tail_split = c.get("tail_split", False)
    mm_deint = c.get("mm_deint", False)
    ramp_in = c.get("ramp_in", False)
    assert not (h_merge and gps), "h_merge owns the full h-stage on DVE"
    assert st_split == 1 or perm_dh, "st_split>1 requires perm_dh"
    chunks_per_split = NCHUNK // st_split

    fp32 = mybir.dt.float32
    bf16 = mybir.dt.bfloat16
    if io_bf16:
        in_dt = bf16
        out_dt = bf16
    else:
        in_dt = mybir.dt.float32r if f32r else fp32
        out_dt = fp32
    # intermediate (PSUM-evict + w-stage) dtype: fp32 keeps the extra
    # rounding steps out of the chain; bf16 halves DVE + ACT byte traffic
    chain_dt = bf16 if c.get("chain_dt", "f32") == "bf16" else fp32
    nc = bacc.Bacc("TRN2", target_bir_lowering=False, debug=False)

    x_d = nc.dram_tensor("x", [SLABS, D, H * WID], in_dt, kind="ExternalInput")
    b_d = nc.dram_tensor("bmat", [128, 128], in_dt, kind="ExternalInput")
    y_d = nc.dram_tensor("y", [SLABS, 8, DH, HH, WH], out_dt, kind="ExternalOutput")

    x_ap = x_d.ap()
    y_ap = y_d.ap()

    with tile.TileContext(nc) as tc:
        with ExitStack() as ctx:
            const_pool = ctx.enter_context(tc.tile_pool(name="const", bufs=1))
            x_pool = ctx.enter_context(tc.tile_pool(name="xin", bufs=x_bufs))
            c_pool = ctx.enter_context(tc.tile_pool(name="cpy", bufs=cw_bufs))
            w_pool = ctx.enter_context(tc.tile_pool(name="wtmp", bufs=cw_bufs))
            st_pool = ctx.enter_context(tc.tile_pool(name="stage", bufs=st_bufs))
            psum_pool = ctx.enter_context(
                tc.tile_pool(name="psum", bufs=2, space="PSUM")
            )

            bt = const_pool.tile([128, 128], in_dt)
            nc.sync.dma_start(bt[:], b_d.ap()[:])

            def body(_i=None):
                for s in range(SLABS):
                    # staging tiles per (b, c) filter pair and hh-split;
                    # the last slab may split finer to shrink the drain burst
                    s_split = (
                        st_split * 2
                        if (tail_split and s == SLABS - 1)
                        else st_split
                    )
                    s_cps = NCHUNK // s_split
                    stf = HH * WH // s_split
                    sts = {}
                    svs = {}
                    stm = {}
                    tl = "_t" if s_split != st_split else ""
                    for hf in range(s_split):
                        if h_merge:
                            # one merged tile per split: free = bc*stf+hh*WH+wh
                            t = st_pool.tile(
                                [128, 4 * stf], out_dt, tag=f"stm_{hf}{tl}",
                                name=f"stm_{hf}{tl}",
                            )
                            stm[hf] = t
                            tap = t[:]
                            for bc in range(4):
                                sts[bc, hf] = tap[:, bc * stf : (bc + 1) * stf]
                            continue
                        for bc in range(4):
                            t = st_pool.tile(
                                [128, stf], out_dt, tag=f"st{bc}_{hf}{tl}",
                                name=f"st{bc}_{hf}{tl}",
                            )
                            sts[bc, hf] = t
                            svs[bc, hf] = t.rearrange(
                                "p (hh wh) -> p hh wh", wh=WH
                            )
                    if perm_dh:
                        # staging partition p = 2*dh + a
                        yvs = y_ap[s].rearrange(
                            "(a b c) dh (hf hh) wh -> (b c) hf dh a (hh wh)",
                            a=2, b=2, c=2, hf=s_split,
                        )
                    # batch schedule: slab 0 ramps in with small first loads
                    # so the compute pipeline fills sooner
                    if ramp_in and s == 0:
                        sched = []
                        rem = NCHUNK
                        for b in (1, 1, 2):
                            sched.append(b)
                            rem -= b
                        while rem > 0:
                            sched.append(min(in_batch, rem))
                            rem -= sched[-1]
                    else:
                        sched = [in_batch] * (NCHUNK // in_batch)
                    q2b = {}
                    base = 0
                    for bi, blen in enumerate(sched):
                        for qo_ in range(blen):
                            q2b[base + qo_] = (bi, qo_, blen, base)
                        base += blen
                    xts = {}
                    for q in range(NCHUNK):
                        qb, qo, blen, qbase = q2b[q]
                        if qo == 0:
                            xtb = x_pool.tile(
                                [128, FREE * blen], in_dt,
                                tag=f"xt{blen}", name=f"xt{blen}",
                            )
                            xts[qb] = xtb
                            nc.sync.dma_start(
                                xtb[:],
                                x_ap[s][
                                    :,
                                    qbase * FREE : (qbase + blen) * FREE,
                                ],
                            )
                        xt = xts[qb][:, qo * FREE : (qo + 1) * FREE]
                        pt = psum_pool.tile([128, FREE], fp32, tag="pt")
                        if mm_deint:
                            # de-interleave w-pairs in the matmul: strided
                            # rhs views (full-rate: 4 bf16 hits per 16B
                            # SBUF line) write PSUM as (t, h, wh), so the
                            # eviction is one contiguous copy and every
                            # DVE operand downstream is packed.
                            xv = xt.rearrange(
                                "p (h wh t) -> p t h wh", t=2, wh=WH
                            )
                            for t_ in range(2):
                                for h2 in range(2):
                                    j = t_ * 2 + h2
                                    nc.tensor.matmul(
                                        pt[:, j * 512 : (j + 1) * 512],
                                        bt[:],
                                        xv[:, t_, h2 * 8 : (h2 + 1) * 8],
                                        start=True,
                                        stop=True,
                                    )
                        else:
                            for j in range(FREE // 512):
                                nc.tensor.matmul(
                                    pt[:, j * 512 : (j + 1) * 512],
                                    bt[:],
                                    xt[:, j * 512 : (j + 1) * 512],
                                    start=True,
                                    stop=True,
                                )
                        # evict PSUM -> SBUF on the (otherwise idle) scalar
                        # engine: DVE tensor_tensor may read only one PSUM
                        # operand, and the butterflies need two.
                        ct = c_pool.tile([128, FREE], chain_dt, tag="ct", name="ct")
                        wt = w_pool.tile([128, FREE], chain_dt, tag="wt", name="wt")
                        # wtmp free layout: c*(HC*WH) + h*WH + wh
                        wv = wt.rearrange("p (c h wh) -> p c h wh", c=2, wh=WH)
                        if mm_deint:
                            # PSUM is already (t, h, wh): one contiguous
                            # eviction, then fully packed w-stage.
                            nc.scalar.copy(ct[:], pt[:])
                            cv = ct.rearrange("p (t hw) -> p t hw", t=2)
                            wf = wt.rearrange("p (c hw) -> p c hw", c=2)
                            nc.vector.tensor_add(wf[:, 0], cv[:, 0], cv[:, 1])
                            nc.vector.tensor_sub(wf[:, 1], cv[:, 0], cv[:, 1])
                        elif split_evict:
                            # de-interleave the w-pairs during eviction (ACT
                            # cost is ap-size-based, stride-free), so every
                            # DVE operand below is a packed 2-byte SBUF run
                            # and qualifies for the DVE fast path.
                            cv = ct.rearrange("p (t hw) -> p t hw", t=2)
                            ptv = pt.rearrange(
                                "p (h wh t) -> p t (h wh)", t=2, wh=WH
                            )
                            nc.scalar.copy(cv[:, 0], ptv[:, 0])
                            nc.scalar.copy(cv[:, 1], ptv[:, 1])
                            wf = wt.rearrange("p (c hw) -> p c hw", c=2)
                            nc.vector.tensor_add(wf[:, 0], cv[:, 0], cv[:, 1])
                            nc.vector.tensor_sub(wf[:, 1], cv[:, 0], cv[:, 1])
                        else:
                            nc.scalar.copy(ct[:], pt[:])
                            # w-axis butterfly: free index h*128 + wh*2 + t
                            pv = ct.rearrange(
                                "p (h wh t) -> p t h wh", t=2, wh=WH
                            )
                            nc.vector.tensor_add(wv[:, 0], pv[:, 0], pv[:, 1])
                            nc.vector.tensor_sub(wv[:, 1], pv[:, 0], pv[:, 1])
                        # h-axis butterfly: h = 2*hh_local + sp
                        hv = wt.rearrange(
                            "p (c hh sp wh) -> p c sp hh wh", sp=2, c=2, wh=WH
                        )
                        hf, ql = divmod(q, s_cps)
                        hh0 = ql * (HC // 2)
                        if h_merge:
                            # one add / one sub spanning both c values: the
                            # merged staging layout (bc, hh, wh) makes the
                            # out AP [cc, hh-slice, wh] expressible
                            mv = stm[hf].rearrange(
                                "p (b c hh wh) -> p b c hh wh",
                                b=2, c=2, wh=WH,
                            )
                            nc.vector.tensor_add(
                                mv[:, 0, :, hh0 : hh0 + HC // 2],
                                hv[:, :, 0],
                                hv[:, :, 1],
                            )
                            nc.vector.tensor_sub(
                                mv[:, 1, :, hh0 : hh0 + HC // 2],
                                hv[:, :, 0],
                                hv[:, :, 1],
                            )
                        else:
                            for cc in range(2):
                                eng = (
                                    nc.gpsimd
                                    if (gps and cc == 1)
                                    else nc.vector
                                )
                                eng.tensor_add(
                                    svs[0 * 2 + cc, hf][:, hh0 : hh0 + HC // 2],
                                    hv[:, cc, 0],
                                    hv[:, cc, 1],
                                )
                                eng.tensor_sub(
                                    svs[1 * 2 + cc, hf][:, hh0 : hh0 + HC // 2],
                                    hv[:, cc, 0],
                                    hv[:, cc, 1],
                                )
                        if q % s_cps == s_cps - 1:
                            st_eng = getattr(nc, st_eng_name)
                            # this hh-split of all 4 staging tiles is complete
                            if perm_dh:
                                for bc in range(4):
                                    src = (
                                        sts[bc, hf]
                                        if h_merge
                                        else sts[bc, hf][:]
                                    )
                                    st_eng.dma_start(yvs[bc][hf], src)
                            else:
                                yv = y_ap[s].rearrange(
                                    "(a b c) dh hh wh -> (b c) a dh (hh wh)",
                                    a=2, b=2, c=2,
                                )
                                for bc in range(4):
                                    for a in range(2):
                                        st_eng.dma_start(
                                            yv[bc][a],
                                            sts[bc, hf][64 * a : 64 * (a + 1)],
                                        )

            if n_iters == 1:
                body()
            else:
                with tc.For_i(0, n_iters, 1) as i:
                    body(i)

    nc.compile()
    nc._haar_cfg = c
    return nc


_CACHED_NC = None


def _get_nc():
    global _CACHED_NC
    if _CACHED_NC is None:
        _CACHED_NC = build_module(1)
    return _CACHED_NC


def _numpy_fallback(x: np.ndarray, w: np.ndarray) -> np.ndarray:
    n, c, d, h, wd = x.shape
    xb = x.reshape(n, c, d // 2, 2, h // 2, 2, wd // 2, 2)
    y = np.einsum("ncdihjwk,oijk->ncodhw", xb, w)
    return y.reshape(n, c * 8, d // 2, h // 2, wd // 2).astype(x.dtype)


def make_in_maps(x: np.ndarray, cfg: dict | None = None) -> list[dict]:
    c = dict(CFG)
    if cfg:
        c.update(cfg)
    bmat = _butterfly_lhsT(c["perm_dh"])
    if c.get("io_dt", "f32") == "bf16":
        import ml_dtypes

        x = x.astype(ml_dtypes.bfloat16)
        bmat = bmat.astype(ml_dtypes.bfloat16)
    xf = x.reshape(32, D, H * WID)
    return [
        {"x": xf[SLABS * k : SLABS * (k + 1)], "bmat": bmat} for k in range(N_CORES)
    ]


def kernel(x: np.ndarray, W: np.ndarray) -> np.ndarray:
    from concourse import bass_utils

    x = np.asarray(x)
    W = np.asarray(W)
    if not np.allclose(W, _haar_weight_np(), rtol=0, atol=1e-12):
        # The butterfly factorization is specialized to the exact Haar bank.
        return _numpy_fallback(x, W)

    n, c, d, h, wd = x.shape
    assert (n, c, d, h, wd) == (2, 16, 128, 128, 128), x.shape

    nc = _get_nc()
    in_maps = make_in_maps(x, nc._haar_cfg)
    res = bass_utils.run_bass_kernel_spmd(nc, in_maps, core_ids=list(range(N_CORES)))
    y = np.stack([np.asarray(res.results[k]["y"]) for k in range(N_CORES)])
    # [8, 4, 8, dh, hh, wh] -> [2, 16, 8, dh, hh, wh] -> [2, 128, dh, hh, wh]
    return y.astype(np.float32).reshape(2, 128, DH, HH, WH)


if __name__ == "__main__":
    rng = np.random.default_rng(0)
    x = rng.standard_normal((2, 16, 128, 128, 128), dtype=np.float32)
    w = _haar_weight_np()
    out = kernel(x, w)
    exp = _numpy_fallback(x, w)
    err = np.abs(out - exp).max() / np.abs(exp).max()
    print("rel err vs numpy:", err)

